# revision 29
# baseline (speedup 1.0000x reference)
"""nn_BayesianLayer — reparameterized Bayesian linear layer + inverted dropout
on 8 TRN2 NeuronCores (data-parallel over the 65536-row batch).

reference:
  w = w_mu + softplus(w_rho) * w_eps            [512, 512]
  b = b_mu + softplus(b_rho) * b_eps            [512]
  y = (x @ w.T + b) * (drop_u >= 0.2) / 0.8     [65536, 512]

Sharding: x and drop_u split into 8 row-shards of 8192; the small weight
tensors are replicated. Each core runs the same single-core Bass/Tile graph
(SPMD, no collectives); outputs are concatenated on the host.

The problem is HBM-bandwidth bound (~358 GB/s per core).  All DRAM tensors
are fp16 (host casts + layout prep): 26.9 MB per core vs 53.8 MB for fp32,
a ~75 us DMA floor.  Measured host-side, the full fp16 pipeline lands at
rel err ~4.2e-3 (vs the 2e-2 gate), dominated by ~433 dropout-mask flips
where drop_u rounds across the 0.2 threshold.

Per-core kernel design (all measured on HW against alternatives):
 - TRANSPOSED GEMM: psum tiles are [128 out-channels, 512 rows], i.e.
   lhsT = w'T chunk [128k, 128oc] (stationary), rhs = xT chunk [128k,
   512 rows] (moving).  This makes the bias a PER-PARTITION scalar, so the
   ACT engine adds it for free during the PSUM->SBUF drain
   (Identity activation, bias=AP): no bias matmul on PE (a K=1 bias matmul
   costs ~670 ns and a K=128 one ~200 ns per output tile on HW — PE is the
   secondary bottleneck at ~300 ns per N=512 matmul, so 256 matmuls just
   fit under the DMA floor while 320 do not), and no second DVE op
   (the 2-op DVE epilogue variant measured +22 us).
 - dropout mask: one fused DVE op per tile, out = (du >= C) * t1 with all
   operands fp16/SBUF (2x_1P DVE mode), C = 0.2000732421875 — the smallest
   fp16 above 0.2, minimizing threshold disagreement vs the fp32 ref.
 - matmuls run fp16 x fp16 -> fp32 PSUM (same PE column rate as bf16);
   x, drop_u and y are streamed transposed ([feature, row]) so each
   per-partition DMA line is >=4 KB contiguous DRAM.
 - prologue computes w'T = 1.25*(w_mu + softplus(w_rho)*w_eps).T on-device
   from fp16 inputs; softplus(rho) for rho in [-3.5, -2.5] uses the 3-term
   series t - t^2/2 + t^3/3, t = exp(rho) (max rel err ~2e-4 there), with
   the 1.25 dropout scale folded in:  sp' = 1.25*t + t^2*(t*(1.25/3) -
   0.625).  ACT does exp/square, GPSIMD + DVE the polynomial/fma tail.
 - batch processed in 4 slabs of 2048 rows; loads (x, drop_u) all ride the
   SP HWDGE ring, stores (y) + weight loads the ACT ring: one ring alone
   can saturate HBM, and this way a store waiting on compute never
   head-of-line-blocks the next slab's loads (matters most at the timing
   loop's back-edge).
"""

import numpy as np

import concourse.bass as bass
import concourse.mybir as mybir
from concourse import bacc, tile
from concourse.bass import ts
from concourse.bass_utils import run_bass_kernel_spmd

AF = mybir.ActivationFunctionType
ALU = mybir.AluOpType

N_CORES = 8
B, IN, OUT = 65536, 512, 512
BS = B // N_CORES          # 8192 rows per core
P = 128
KC = IN // P               # 4 contraction chunks
T = OUT // P               # 4 output-channel tiles
SLABS = 8                  # batch slabs per core
GB = BS // SLABS           # 2048 rows per slab
RC = GB // 512             # 4 row-chunks (N=512 matmuls) per slab
DROP_C = 0.2000732421875   # smallest fp16 strictly above 0.2
SCALE = 1.25               # 1/(1-0.2), folded into w', b'


def build_kernel(x_bufs=4, du_bufs=4, out_bufs=3, psum_bufs=8, reps=1,
                 n_k=KC):
    # NB: gpsimd/Pool cannot lower TensorScalarPtr (NCC_IXCG966) — the
    # masked-mult stt must run on DVE; only plain TensorTensor works on Pool.
    import contextlib
    nc = bacc.Bacc(None, target_bir_lowering=False, debug=False)
    f32 = mybir.dt.float32
    f16 = mybir.dt.float16

    xh = nc.declare_dram_parameter("xh", [P, SLABS * KC * GB], f16, isOutput=False)
    wmu = nc.declare_dram_parameter("wmu", [IN, OUT], f16, isOutput=False)
    wrho = nc.declare_dram_parameter("wrho", [IN, OUT], f16, isOutput=False)
    weps = nc.declare_dram_parameter("weps", [IN, OUT], f16, isOutput=False)
    bmu = nc.declare_dram_parameter("bmu", [OUT, 1], f16, isOutput=False)
    brho = nc.declare_dram_parameter("brho", [OUT, 1], f16, isOutput=False)
    beps = nc.declare_dram_parameter("beps", [OUT, 1], f16, isOutput=False)
    # slab-major [p, s, t, b] layouts: per partition, one slab's worth of
    # drop_u / y is a single contiguous 16 KB DRAM line
    duh = nc.declare_dram_parameter("duh", [P, SLABS * T * GB], f16,
                                    isOutput=False)
    yh = nc.declare_dram_parameter("yh", [P, SLABS * T * GB], f16,
                                   isOutput=True)

    xh_r = xh[:, :].rearrange("p (s k r) -> p s k r", s=SLABS, k=KC)
    wmu_r = wmu[:, :].rearrange("(k p) n -> p k n", p=P)
    wrho_r = wrho[:, :].rearrange("(k p) n -> p k n", p=P)
    weps_r = weps[:, :].rearrange("(k p) n -> p k n", p=P)
    bmu_r = bmu[:, :].rearrange("(t p) o -> p t o", p=P)
    brho_r = brho[:, :].rearrange("(t p) o -> p t o", p=P)
    beps_r = beps[:, :].rearrange("(t p) o -> p t o", p=P)
    duh_r = duh[:, :].rearrange("p (s t b) -> p s t b", s=SLABS, t=T)
    yh_r = yh[:, :].rearrange("p (s t b) -> p s t b", s=SLABS, t=T)

    with tile.TileContext(nc) as tc:
        with (
            tc.tile_pool(name="wt", bufs=1) as wt_pool,
            tc.tile_pool(name="prol", bufs=2) as prol_pool,
            tc.tile_pool(name="bias", bufs=1) as bias_pool,
            tc.tile_pool(name="xs", bufs=x_bufs) as x_pool,
            tc.tile_pool(name="dus", bufs=du_bufs) as du_pool,
            tc.tile_pool(name="outs", bufs=out_bufs) as out_pool,
            tc.tile_pool(name="t1", bufs=4) as t1_pool,
            tc.tile_pool(name="ps", bufs=psum_bufs, space="PSUM") as psum_pool,
        ):
            def emit_sp125(dst, rho_t, pool, shape):
                """dst(f32) = 1.25*softplus(rho_t), 3-term exp series."""
                t = pool.tile(shape, f32, tag="t")
                t2 = pool.tile(shape, f32, tag="t2")
                a = pool.tile(shape, f32, tag="a")
                nc.scalar.activation(t[:], rho_t[:], AF.Exp)
                nc.scalar.square(t2[:], t[:])
                nc.gpsimd.tensor_scalar(a[:], t[:], SCALE / 3.0, -0.625,
                                        ALU.mult, ALU.add)
                nc.gpsimd.tensor_mul(a[:], t2[:], a[:])
                # dst = 1.25*t + t^2*(t*(1.25/3) - 0.625)
                nc.vector.scalar_tensor_tensor(
                    dst[:], t[:], SCALE, a[:], ALU.mult, ALU.add)

            # ---- weight prologue, per-k chunk (pipelines ACT/GPSIMD/DVE):
            # wt[k] = fp16 w'T chunk [128k, OUT]; lhsT tiles are its
            # [:, t*128:(t+1)*128] slices ----
            wt = []
            for k in range(KC):
                rho_t = prol_pool.tile([P, OUT], f16, tag="rho")
                mu_t = prol_pool.tile([P, OUT], f16, tag="mu")
                eps_t = prol_pool.tile([P, OUT], f16, tag="eps")
                nc.scalar.dma_start(out=rho_t[:], in_=wrho_r[:, k])
                nc.scalar.dma_start(out=mu_t[:], in_=wmu_r[:, k])
                nc.scalar.dma_start(out=eps_t[:], in_=weps_r[:, k])
                sp = prol_pool.tile([P, OUT], f32, tag="sp")
                emit_sp125(sp, rho_t, prol_pool, [P, OUT])
                nc.vector.tensor_mul(sp[:], sp[:], eps_t[:])
                wtk = wt_pool.tile([P, OUT], f16, tag=f"wt{k}")
                # wt = mu*1.25 + sp*eps   (downcast to fp16 on write)
                nc.vector.scalar_tensor_tensor(
                    wtk[:], mu_t[:], SCALE, sp[:], ALU.mult, ALU.add)
                wt.append(wtk)

            # ---- bias prologue: bcol [P, T] f32, bcol[p, t] = 1.25*b'[t*128+p]
            bmu_t = bias_pool.tile([P, T], f16, tag="bmu")
            brho_t = bias_pool.tile([P, T], f16, tag="brho")
            beps_t = bias_pool.tile([P, T], f16, tag="beps")
            nc.scalar.dma_start(out=bmu_t[:], in_=bmu_r[:, :, 0])
            nc.scalar.dma_start(out=brho_t[:], in_=brho_r[:, :, 0])
            nc.scalar.dma_start(out=beps_t[:], in_=beps_r[:, :, 0])
            spb = bias_pool.tile([P, T], f32, tag="spb")
            emit_sp125(spb, brho_t, bias_pool, [P, T])
            nc.vector.tensor_mul(spb[:], spb[:], beps_t[:])
            bcol = bias_pool.tile([P, T], f32, tag="bcol")
            nc.vector.scalar_tensor_tensor(
                bcol[:], bmu_t[:], SCALE, spb[:], ALU.mult, ALU.add)

            # ---- main loop, software-pipelined one slab: loads(s) are
            # emitted before compute(s-1)+store(s-1).  Loads ride the SP
            # ring, stores + weights the ACT ring ----
            hk = KC // 2

            def emit_loads(s):
                xs = x_pool.tile([P, KC, GB], f16, tag="xs")
                dus = du_pool.tile([P, T, GB], f16, tag="dus")
                nc.sync.dma_start(out=xs[:, :hk], in_=xh_r[:, s, :hk])
                nc.sync.dma_start(out=dus[:, :2], in_=duh_r[:, s, :2])
                nc.sync.dma_start(out=xs[:, hk:], in_=xh_r[:, s, hk:])
                nc.sync.dma_start(out=dus[:, 2:], in_=duh_r[:, s, 2:])
                return xs, dus

            def emit_compute_store(s, xs, dus):
                # t outer so each outs[:, t] quarter completes early and its
                # store streams during the slab's remaining compute (no
                # end-of-slab store burst)
                outs = out_pool.tile([P, T, GB], f16, tag="outs")
                for t in range(T):
                    for rc in range(RC):
                        ps = psum_pool.tile([P, 512], f32, tag="ps")
                        for k in range(n_k):
                            nc.tensor.matmul(
                                ps[:], wt[k][:, ts(t, P)],
                                xs[:, k, ts(rc, 512)],
                                start=(k == 0), stop=(k == n_k - 1))
                        # PSUM drain on ACT adds the per-partition bias
                        t1 = t1_pool.tile([P, 512], f16, tag="t1")
                        nc.scalar.activation(t1[:], ps[:], AF.Identity,
                                             bias=bcol[:, ts(t, 1)])
                        # out = (drop_u >= C) * t1   (one fused DVE op,
                        # all-fp16 SBUF operands -> 2x mode)
                        nc.vector.scalar_tensor_tensor(
                            outs[:, t, ts(rc, 512)], dus[:, t, ts(rc, 512)],
                            DROP_C, t1[:], ALU.is_ge, ALU.mult)
                    nc.scalar.dma_start(out=yh_r[:, s, t], in_=outs[:, t])

            reps_cm = (tc.For_i(0, reps, name="reps", staggered_reset=True)
                       if reps > 1 else contextlib.nullcontext())
            with reps_cm:
                prev = None
                for s in range(SLABS):
                    cur = emit_loads(s)
                    if prev is not None:
                        emit_compute_store(s - 1, *prev)
                    prev = cur
                emit_compute_store(SLABS - 1, *prev)

    nc.finalize()
    return nc


def shard_inputs(x, w_mu, w_rho, b_mu, b_rho, w_eps, b_eps, drop_u):
    """Full inputs -> per-core in_maps (host-side cast + layout prep)."""
    f16 = np.float16
    wmu_t = np.ascontiguousarray(np.asarray(w_mu).T.astype(f16))
    wrho_t = np.ascontiguousarray(np.asarray(w_rho).T.astype(f16))
    weps_t = np.ascontiguousarray(np.asarray(w_eps).T.astype(f16))
    bmu = np.asarray(b_mu, f16).reshape(OUT, 1)
    brho = np.asarray(b_rho, f16).reshape(OUT, 1)
    beps = np.asarray(b_eps, f16).reshape(OUT, 1)
    x = np.asarray(x)
    drop_u = np.asarray(drop_u)
    in_maps = []
    for c in range(N_CORES):
        sl = slice(c * BS, (c + 1) * BS)
        # xh[p, s, k, r] = x[c*BS + s*GB + r, k*P + p]
        x5 = x[sl].astype(f16).reshape(SLABS, GB, KC, P)
        xh = np.ascontiguousarray(x5.transpose(3, 0, 2, 1)).reshape(
            P, SLABS * KC * GB)
        # duh[p, s, t, b] = drop_u[c*BS + s*GB + b, t*P + p]
        du4 = drop_u[sl].astype(f16).reshape(SLABS, GB, T, P)
        duh = np.ascontiguousarray(du4.transpose(3, 0, 2, 1)).reshape(
            P, SLABS * T * GB)
        in_maps.append({
            "xh": xh,
            "wmu": wmu_t, "wrho": wrho_t, "weps": weps_t,
            "bmu": bmu, "brho": brho, "beps": beps,
            "duh": duh,
        })
    return in_maps


def unpack_y(yh):
    """yh [P, SLABS*T*GB] (slab-major) -> y [BS, OUT] fp32 for one core."""
    return yh.reshape(P, SLABS, T, GB).transpose(1, 3, 2, 0).reshape(BS, OUT)


def kernel(x, w_mu, w_rho, b_mu, b_rho, w_eps, b_eps, drop_u):
    nc = build_kernel()
    in_maps = shard_inputs(x, w_mu, w_rho, b_mu, b_rho, w_eps, b_eps, drop_u)
    res = run_bass_kernel_spmd(nc, in_maps, core_ids=list(range(N_CORES)))
    return np.ascontiguousarray(np.concatenate(
        [unpack_y(res.results[c]["yh"]) for c in range(N_CORES)], axis=0)
    ).astype(np.float32)
